# revision 20
# baseline (speedup 1.0000x reference)
"""Trainium2 Bass kernel for 3D attention with decomposed rel-pos bias.

Problem: nn_Attention3D (B=2, D=8, H=16, W=16, C=768, nh=12, hd=64).

Strategy
--------
- Shard the 24 (batch, head) pairs as 3 per core x 8 cores; each core's 3
  heads share one batch, so the core can emit a single per-batch partial of
  the output projection (summed on host across the 4 cores per batch).
- Rel-pos bias is folded into the QK^T matmul by augmenting the contraction
  dim: score[q,k] = [scale*q, U_d(q), U_h(q), U_w(q)] . [k, 1hot_d(k),
  1hot_h(k), 1hot_w(k)], with U_d = q @ Rd[d(q)]^T etc.  64+8+16+16 = 104
  <= 128 partitions, so the whole biased logit matrix costs one matmul pass
  and zero elementwise broadcast adds.
- Softmax without max-subtraction (logits are ~N(0, 0.3), |logit| < 3).
- Softmax denominator folded into the PV matmul as a ones-column of V.
- All matmuls in float32r (full PE speed at N>=256), fp32 data everywhere.
"""

import os
import sys

import numpy as np
import ml_dtypes

for _p in ("/opt/trn_rl_repo", "/root/.axon_site/_ro/trn_rl_repo"):
    if os.path.isdir(_p) and _p not in sys.path:
        sys.path.append(_p)

import concourse.bass as bass
import concourse.mybir as mybir
from concourse import bacc
from concourse.bass_utils import run_bass_kernel_spmd
from concourse.tile import TileContext

F32 = mybir.dt.float32
F32R = mybir.dt.float32r
BF16 = mybir.dt.bfloat16
EXP = mybir.ActivationFunctionType.Exp

B, D, HH, WW, C = 2, 8, 16, 16, 768
NH, HD = 12, 64
N = D * HH * WW            # 2048
SCALE = HD ** -0.5
NCORES = 8
HPC = 3                    # heads per core
KT = C // 128              # 6 contraction tiles for the input projections
AUG = HD + D + HH + WW     # 104 augmented contraction

LAST_RESULTS = None        # stashed BassKernelResults for test.py


def _build_bass():
    nc = bacc.Bacc()
    xT_d = nc.dram_tensor("xT", [128, KT, N], BF16, kind="ExternalInput")
    wqk_d = nc.dram_tensor("w_qk", [128, HPC, KT, 128], BF16, kind="ExternalInput")
    bqk_d = nc.dram_tensor("b_qk", [128, HPC], F32, kind="ExternalInput")
    wv_d = nc.dram_tensor("w_v", [128, KT, 256], BF16, kind="ExternalInput")
    bv_d = nc.dram_tensor("b_v", [128, 256], F32, kind="ExternalInput")
    rd_d = nc.dram_tensor("rd_t", [64, D, D], BF16, kind="ExternalInput")
    rh_d = nc.dram_tensor("rh_t", [64, HH, HH], BF16, kind="ExternalInput")
    rw_d = nc.dram_tensor("rw_t", [64, WW, WW], BF16, kind="ExternalInput")
    oneh_d = nc.dram_tensor("onehot", [40, N], BF16, kind="ExternalInput")
    wp_d = nc.dram_tensor("w_p", [64, HPC, C], BF16, kind="ExternalInput")
    ones_d = nc.dram_tensor("ones_col", [128, 16, 64], BF16, kind="ExternalInput")
    out_d = nc.dram_tensor("out_part", [N, C], F32, kind="ExternalOutput")

    with TileContext(nc) as tc:
        with (
            tc.tile_pool(name="const", bufs=1) as const,
            tc.tile_pool(name="work", bufs=2) as work,
            tc.tile_pool(name="vpool", bufs=1) as vpool,
            tc.tile_pool(name="ptp", bufs=2) as ptp,
            tc.tile_pool(name="aout", bufs=1) as aout,
            tc.tile_pool(name="ustage", bufs=2) as ustage,
            tc.tile_pool(name="small", bufs=1) as small,
            tc.tile_pool(name="outp", bufs=2) as outp,
        ):
            # ---- constants: attention-critical DMAs first ----
            xT = const.tile([128, KT, N], BF16, tag="xT")
            for kt in range(KT):
                nc.sync.dma_start(out=xT[:, kt, :], in_=xT_d[:, kt, :])
            wv = const.tile([128, KT, 256], BF16, tag="wv")
            nc.sync.dma_start(out=wv, in_=wv_d[:, :, :])
            bv = const.tile([128, 256], F32, tag="bv")
            nc.sync.dma_start(out=bv, in_=bv_d[:, :])
            wqk = const.tile([128, HPC, KT, 128], BF16, tag="wqk")
            nc.sync.dma_start(out=wqk, in_=wqk_d[:, :, :, :])
            bqk = const.tile([128, HPC], F32, tag="bqk")
            nc.sync.dma_start(out=bqk, in_=bqk_d[:, :])
            rd = const.tile([64, D, D], BF16, tag="rd")
            nc.sync.dma_start(out=rd, in_=rd_d[:, :, :])
            rh = const.tile([64, HH, HH], BF16, tag="rh")
            nc.sync.dma_start(out=rh, in_=rh_d[:, :, :])
            rw = const.tile([64, WW, WW], BF16, tag="rw")
            nc.sync.dma_start(out=rw, in_=rw_d[:, :, :])
            wp = const.tile([64, HPC, C], BF16, tag="wp")
            nc.sync.dma_start(out=wp, in_=wp_d[:, :, :])

            # Prime the natural_log_exp ACT table set (has BOTH Exp and Ln)
            # so no mid-kernel table switches happen: Ln(0*x + 1) = 0.
            LN = mybir.ActivationFunctionType.Ln
            dummy = small.tile([1, 1], F32, tag="dummy")
            nc.scalar.activation(dummy, bqk[0:1, 0:1], LN, scale=0.0, bias=1.0)

            V = []
            for hl in range(HPC):
                vt = vpool.tile([128, 16, 128], BF16, tag=f"v{hl}")
                nc.sync.dma_start(out=vt[:, :, 64:128], in_=ones_d[:, :, :])
                V.append(vt)
            AOUT = [aout.tile([64, N], BF16, tag=f"a{hl}", name=f"aout{hl}")
                    for hl in range(HPC)]
            ANRM = [aout.tile([64, N], BF16, tag=f"an{hl}", name=f"anrm{hl}")
                    for hl in range(HPC)]
            DEN = [aout.tile([64, N], F32, tag=f"dn{hl}", name=f"den{hl}")
                    for hl in range(HPC)]

            with tc.tile_pool(name="psA", bufs=2, space="PSUM") as psA:
                # ---- V projection (all 3 heads at once) ----
                for tt in range(16):
                    psv = psA.tile([128, 256], F32, tag="mm")
                    for kt in range(KT):
                        nc.tensor.matmul(
                            psv,
                            xT[:, kt, tt * 128:(tt + 1) * 128],
                            wv[:, kt, :],
                            start=(kt == 0), stop=(kt == KT - 1),
                        )
                    for hl in range(HPC):
                        nc.vector.tensor_add(
                            V[hl][:, tt, 0:64],
                            psv[:, hl * 64:(hl + 1) * 64],
                            bv[:, hl * 64:(hl + 1) * 64],
                        )

                def qku_steps(hl):
                    """Build Qa/Ka for head hl as a list of small closures,
                    interleaved one per kt-slot into the previous head's
                    attention so the PE fills exp-shadow idle time."""
                    Qa = work.tile([AUG, N], BF16, tag="qaug", name=f"qa{hl}")
                    Ka = work.tile([AUG, N], BF16, tag="kaug", name=f"ka{hl}")
                    st_d = ustage.tile([D, N], BF16, tag="ud", name=f"ud{hl}")
                    st_h = ustage.tile([HH, N], BF16, tag="uh", name=f"uh{hl}")
                    st_w = ustage.tile([WW, N], BF16, tag="uw", name=f"uw{hl}")
                    steps = []
                    steps.append(lambda: nc.sync.dma_start(
                        out=Ka[64:104, :], in_=oneh_d[:, :]))

                    def qk_slice(qs):
                        sl = slice(qs * 512, (qs + 1) * 512)
                        pqk = psA.tile([128, 512], F32, tag="mm",
                                       name=f"pqk{hl}_{qs}")
                        for kt in range(KT):
                            nc.tensor.matmul(
                                pqk, wqk[:, hl, kt, :], xT[:, kt, sl],
                                start=(kt == 0), stop=(kt == KT - 1))
                        nc.vector.tensor_scalar_add(
                            Qa[0:64, sl], pqk[0:64, :], bqk[0:64, hl:hl + 1])
                        nc.vector.tensor_scalar_add(
                            Ka[0:64, sl], pqk[64:128, :], bqk[64:128, hl:hl + 1])
                    for qs in range(4):
                        steps.append(lambda qs=qs: qk_slice(qs))

                    def ud_pair(d0):
                        pu = psA.tile([D, 2, 256], F32, tag="mm",
                                      name=f"pud{hl}_{d0}")
                        for s in range(2):
                            nc.tensor.matmul(
                                pu[:, s, :], rd[:, d0 + s, :],
                                Qa[0:64, (d0 + s) * 256:(d0 + s + 1) * 256],
                                start=True, stop=True)
                        nc.vector.tensor_copy(
                            st_d[:, d0 * 256:(d0 + 2) * 256],
                            pu.rearrange("p s n -> p (s n)"))
                    for d0 in range(0, D, 2):
                        steps.append(lambda d0=d0: ud_pair(d0))
                    steps.append(lambda: nc.sync.dma_start(
                        out=Qa[64:72, :], in_=st_d))

                    q_h = Qa[0:64, :].rearrange(
                        "p (d h w) -> p d h w", d=D, h=HH, w=WW)
                    u_h = st_h.rearrange(
                        "p (d h w) -> p d h w", d=D, h=HH, w=WW)

                    def uh_quad(h0):
                        pu = psA.tile([HH, 4, D, WW], F32, tag="mm",
                                      name=f"puh{hl}_{h0}")
                        for s in range(4):
                            nc.tensor.matmul(
                                pu[:, s, :, :], rh[:, h0 + s, :],
                                q_h[:, :, h0 + s, :],
                                start=True, stop=True)
                        # in: [p, s, d, w] -> iterate (d, s, w) to match dst
                        nc.vector.tensor_copy(
                            u_h[:, :, h0:h0 + 4, :],
                            pu.rearrange("p s d w -> p d s w"))
                    for h0 in range(0, HH, 4):
                        steps.append(lambda h0=h0: uh_quad(h0))
                    steps.append(lambda: nc.sync.dma_start(
                        out=Qa[72:88, :], in_=st_h))

                    q_w = Qa[0:64, :].rearrange("p (a w) -> p a w", w=WW)
                    u_w = st_w.rearrange("p (a w) -> p a w", w=WW)

                    def uw_quad(w0):
                        pu = psA.tile([WW, 4, D * HH], F32, tag="mm",
                                      name=f"puw{hl}_{w0}")
                        for s in range(4):
                            nc.tensor.matmul(
                                pu[:, s, :], rw[:, w0 + s, :],
                                q_w[:, :, w0 + s],
                                start=True, stop=True)
                        nc.vector.tensor_copy(
                            u_w[:, :, w0:w0 + 4],
                            pu.rearrange("p s a -> p a s"))
                    for w0 in range(0, WW, 4):
                        steps.append(lambda w0=w0: uw_quad(w0))
                    steps.append(lambda: nc.sync.dma_start(
                        out=Qa[88:104, :], in_=st_w))
                    return (Qa, Ka), steps

                with (
                    tc.tile_pool(name="pss", bufs=2, space="PSUM") as pss,
                    tc.tile_pool(name="pso", bufs=1, space="PSUM") as pso,
                ):
                    # prologue: head 0's Qa/Ka built up front
                    (Qa0, Ka0), steps0 = qku_steps(0)
                    for s in steps0:
                        s()
                    cur = (Qa0, Ka0)
                    pending = []
                    for hl in range(HPC):
                        Qa, Ka = cur
                        if hl + 1 < HPC:
                            nxt, pending = qku_steps(hl + 1)
                        else:
                            nxt, pending = None, []
                        step_i = 0
                        for half in range(2):
                            qo = half * 1024
                            ot = pso.tile([128, 1024], F32, tag="o",
                                          name=f"ot{hl}_{half}")
                            prev = None
                            for kt in range(16):
                                st = pss.tile([128, 1024], F32, tag="s",
                                              name=f"st{hl}_{half}_{kt}")
                                for j in range(2):
                                    nc.tensor.matmul(
                                        st[:, j * 512:(j + 1) * 512],
                                        Ka[:, kt * 128:(kt + 1) * 128],
                                        Qa[:, qo + j * 512: qo + (j + 1) * 512],
                                        start=True, stop=True)
                                if step_i < len(pending):
                                    pending[step_i]()
                                    step_i += 1
                                if prev is not None:
                                    pkt, ppt = prev
                                    for j in range(2):
                                        nc.tensor.matmul(
                                            ot[:, j * 512:(j + 1) * 512],
                                            V[hl][:, pkt, :],
                                            ppt[:, j * 512:(j + 1) * 512],
                                            start=(pkt == 0), stop=False)
                                pt = ptp.tile([128, 1024], BF16, tag="pt",
                                              name=f"pt{hl}_{half}_{kt}")
                                nc.scalar.activation(pt, st, EXP)
                                prev = (kt, pt)
                            pkt, ppt = prev
                            for j in range(2):
                                nc.tensor.matmul(
                                    ot[:, j * 512:(j + 1) * 512],
                                    V[hl][:, pkt, :],
                                    ppt[:, j * 512:(j + 1) * 512],
                                    start=False, stop=True)
                            nc.vector.tensor_copy(
                                AOUT[hl][:, qo:qo + 1024], ot[0:64, :])
                            nc.vector.tensor_copy(
                                DEN[hl][:, qo:qo + 1024], ot[64:128, :])
                        # drain any leftover steps for the next head
                        while step_i < len(pending):
                            pending[step_i]()
                            step_i += 1
                        cur = nxt

            # ---- tail normalize (off critical path) + projection ----
            for hl in range(HPC):
                lnd = small.tile([64, N], F32, tag="lnd")
                nc.scalar.activation(lnd, DEN[hl], LN)
                rcp = small.tile([64, N], F32, tag="rcp")
                nc.scalar.activation(rcp, lnd, EXP, scale=-1.0)
                nc.vector.tensor_mul(ANRM[hl], AOUT[hl][0:64, :], rcp)
            with tc.tile_pool(name="psp", bufs=2, space="PSUM") as psp:
                for tt in range(16):
                    pp = psp.tile([128, C], F32, tag="pp")
                    for hl in range(HPC):
                        nc.tensor.matmul(
                            pp[:, 0:512],
                            ANRM[hl][:, tt * 128:(tt + 1) * 128],
                            wp[:, hl, 0:512],
                            start=(hl == 0), stop=(hl == HPC - 1))
                        nc.tensor.matmul(
                            pp[:, 512:768],
                            ANRM[hl][:, tt * 128:(tt + 1) * 128],
                            wp[:, hl, 512:768],
                            start=(hl == 0), stop=(hl == HPC - 1))
                    osb = outp.tile([128, C], F32, tag="osb")
                    nc.vector.tensor_copy(osb, pp)
                    nc.sync.dma_start(
                        out=out_d[tt * 128:(tt + 1) * 128, :], in_=osb)
    nc.compile()
    return nc


def _get_rel(sz, t):
    coords = np.arange(sz)[:, None] - np.arange(sz)[None, :] + sz - 1
    return t[coords]


def kernel(x, qkv_w, qkv_b, proj_w, proj_b,
           rel_pos_d, rel_pos_h, rel_pos_w):
    global LAST_RESULTS
    x = np.ascontiguousarray(np.asarray(x, dtype=np.float32))
    qkv_w = np.asarray(qkv_w, dtype=np.float32)
    qkv_b = np.asarray(qkv_b, dtype=np.float32)
    proj_w = np.asarray(proj_w, dtype=np.float32)
    proj_b = np.asarray(proj_b, dtype=np.float32)
    rel_pos_d = np.asarray(rel_pos_d, dtype=np.float32)
    rel_pos_h = np.asarray(rel_pos_h, dtype=np.float32)
    rel_pos_w = np.asarray(rel_pos_w, dtype=np.float32)

    xf = x.reshape(B, N, C)

    # shared host-prepped tables
    rd_t = np.ascontiguousarray(
        (_get_rel(D, rel_pos_d) / SCALE).transpose(2, 0, 1), np.float32)
    rh_t = np.ascontiguousarray(
        (_get_rel(HH, rel_pos_h) / SCALE).transpose(2, 0, 1), np.float32)
    rw_t = np.ascontiguousarray(
        (_get_rel(WW, rel_pos_w) / SCALE).transpose(2, 0, 1), np.float32)
    dd = np.arange(N) // (HH * WW)
    hh = (np.arange(N) // WW) % HH
    ww = np.arange(N) % WW
    onehot = np.zeros((40, N), np.float32)
    onehot[dd, np.arange(N)] = 1.0
    onehot[D + hh, np.arange(N)] = 1.0
    onehot[D + HH + ww, np.arange(N)] = 1.0

    xT_b = []
    for b in range(B):
        xT = xf[b].T.reshape(KT, 128, N).transpose(1, 0, 2)
        xT_b.append(np.ascontiguousarray(xT))

    in_maps = []
    for core in range(NCORES):
        b = core // 4
        heads = [HPC * (core % 4) + i for i in range(HPC)]
        wqk_l, bqk_l, wv_l, bv_l, wp_l = [], [], [], [], []
        for h in heads:
            wqk_l.append(np.concatenate([
                qkv_w[:, h * HD:(h + 1) * HD] * SCALE,
                qkv_w[:, C + h * HD: C + (h + 1) * HD]], axis=1))
            bqk_l.append(np.concatenate([
                qkv_b[h * HD:(h + 1) * HD] * SCALE,
                qkv_b[C + h * HD: C + (h + 1) * HD]]))
            wv_l.append(qkv_w[:, 2 * C + h * HD: 2 * C + (h + 1) * HD])
            bv_l.append(qkv_b[2 * C + h * HD: 2 * C + (h + 1) * HD])
            wp_l.append(proj_w[h * HD:(h + 1) * HD, :])
        w_qk = np.stack(wqk_l)                       # [3, 768, 128]
        w_qk = np.ascontiguousarray(
            w_qk.reshape(HPC, KT, 128, 128).transpose(2, 0, 1, 3))
        b_qk = np.ascontiguousarray(np.stack(bqk_l).T)   # [128, 3]
        w_v = np.concatenate(wv_l + [np.zeros((C, 64), np.float32)], axis=1)
        w_v = np.ascontiguousarray(
            w_v.reshape(KT, 128, 256).transpose(1, 0, 2))
        b_v = np.broadcast_to(
            np.concatenate(bv_l + [np.zeros(64, np.float32)])[None, :],
            (128, 256))
        w_p = np.ascontiguousarray(np.stack(wp_l).transpose(1, 0, 2))

        bf = ml_dtypes.bfloat16
        in_maps.append({
            "xT": xT_b[b].astype(bf),
            "w_qk": w_qk.astype(bf),
            "b_qk": b_qk,
            "w_v": w_v.astype(bf),
            "b_v": np.ascontiguousarray(b_v),
            "rd_t": rd_t.astype(bf),
            "rh_t": rh_t.astype(bf),
            "rw_t": rw_t.astype(bf),
            "onehot": onehot.astype(bf),
            "w_p": w_p.astype(bf),
            "ones_col": np.ones((128, 16, 64), bf),
        })

    nc = _build_bass()
    res = run_bass_kernel_spmd(nc, in_maps, core_ids=list(range(NCORES)))
    LAST_RESULTS = res

    out = np.zeros((B, N, C), np.float32)
    for core in range(NCORES):
        out[core // 4] += res.results[core]["out_part"]
    out += proj_b
    return out.reshape(B, D, HH, WW, C)


# revision 21
# speedup vs baseline: 1.1279x; 1.1279x over previous
"""Trainium2 Bass kernel for 3D attention with decomposed rel-pos bias.

Problem: nn_Attention3D (B=2, D=8, H=16, W=16, C=768, nh=12, hd=64).

Strategy
--------
- Shard the 24 (batch, head) pairs as 3 per core x 8 cores; each core's 3
  heads share one batch, so the core can emit a single per-batch partial of
  the output projection (summed on host across the 4 cores per batch).
- Rel-pos bias is folded into the QK^T matmul by augmenting the contraction
  dim: score[q,k] = [scale*q, U_d(q), U_h(q), U_w(q)] . [k, 1hot_d(k),
  1hot_h(k), 1hot_w(k)], with U_d = q @ Rd[d(q)]^T etc.  64+8+16+16 = 104
  <= 128 partitions, so the whole biased logit matrix costs one matmul pass
  and zero elementwise broadcast adds.
- Softmax without max-subtraction (logits are ~N(0, 0.3), |logit| < 3).
- Softmax denominator folded into the PV matmul as a ones-column of V.
- All matmuls in float32r (full PE speed at N>=256), fp32 data everywhere.
"""

import os
import sys

import numpy as np
import ml_dtypes

for _p in ("/opt/trn_rl_repo", "/root/.axon_site/_ro/trn_rl_repo"):
    if os.path.isdir(_p) and _p not in sys.path:
        sys.path.append(_p)

import concourse.bass as bass
import concourse.mybir as mybir
from concourse import bacc
from concourse.bass_utils import run_bass_kernel_spmd
from concourse.tile import TileContext

F32 = mybir.dt.float32
F32R = mybir.dt.float32r
BF16 = mybir.dt.bfloat16
EXP = mybir.ActivationFunctionType.Exp

B, D, HH, WW, C = 2, 8, 16, 16, 768
NH, HD = 12, 64
N = D * HH * WW            # 2048
SCALE = HD ** -0.5
NCORES = 8
HPC = 3                    # heads per core
KT = C // 128              # 6 contraction tiles for the input projections
AUG = HD + D + HH + WW     # 104 augmented contraction

LAST_RESULTS = None        # stashed BassKernelResults for test.py


def _build_bass():
    nc = bacc.Bacc()
    xT_d = nc.dram_tensor("xT", [128, KT, N], BF16, kind="ExternalInput")
    wqk_d = nc.dram_tensor("w_qk", [128, HPC, KT, 128], BF16, kind="ExternalInput")
    bqk_d = nc.dram_tensor("b_qk", [128, HPC], F32, kind="ExternalInput")
    wv_d = nc.dram_tensor("w_v", [128, KT, 256], BF16, kind="ExternalInput")
    bv_d = nc.dram_tensor("b_v", [128, 256], F32, kind="ExternalInput")
    rd_d = nc.dram_tensor("rd_t", [64, D, D], BF16, kind="ExternalInput")
    rh_d = nc.dram_tensor("rh_t", [64, HH, HH], BF16, kind="ExternalInput")
    rw_d = nc.dram_tensor("rw_t", [64, WW, WW], BF16, kind="ExternalInput")
    oneh_d = nc.dram_tensor("onehot", [40, N], BF16, kind="ExternalInput")
    wp_d = nc.dram_tensor("w_p", [64, HPC, C], BF16, kind="ExternalInput")
    ones_d = nc.dram_tensor("ones_col", [128, 16, 64], BF16, kind="ExternalInput")
    out_d = nc.dram_tensor("out_part", [N, C], F32, kind="ExternalOutput")

    with TileContext(nc) as tc:
        with (
            tc.tile_pool(name="const", bufs=1) as const,
            tc.tile_pool(name="work", bufs=2) as work,
            tc.tile_pool(name="vpool", bufs=1) as vpool,
            tc.tile_pool(name="ptp", bufs=2) as ptp,
            tc.tile_pool(name="aout", bufs=1) as aout,
            tc.tile_pool(name="ustage", bufs=2) as ustage,
            tc.tile_pool(name="small", bufs=1) as small,
            tc.tile_pool(name="outp", bufs=2) as outp,
        ):
            # ---- constants: attention-critical DMAs first ----
            xT = const.tile([128, KT, N], BF16, tag="xT")
            for tt in range(16):
                sl = slice(tt * 128, (tt + 1) * 128)
                nc.sync.dma_start(out=xT[:, :, sl], in_=xT_d[:, :, sl])
            wv = const.tile([128, KT, 256], BF16, tag="wv")
            nc.sync.dma_start(out=wv, in_=wv_d[:, :, :])
            bv = const.tile([128, 256], F32, tag="bv")
            nc.sync.dma_start(out=bv, in_=bv_d[:, :])
            wqk = const.tile([128, HPC, KT, 128], BF16, tag="wqk")
            nc.sync.dma_start(out=wqk, in_=wqk_d[:, :, :, :])
            bqk = const.tile([128, HPC], F32, tag="bqk")
            nc.sync.dma_start(out=bqk, in_=bqk_d[:, :])
            rd = const.tile([64, D, D], BF16, tag="rd")
            nc.sync.dma_start(out=rd, in_=rd_d[:, :, :])
            rh = const.tile([64, HH, HH], BF16, tag="rh")
            nc.sync.dma_start(out=rh, in_=rh_d[:, :, :])
            rw = const.tile([64, WW, WW], BF16, tag="rw")
            nc.sync.dma_start(out=rw, in_=rw_d[:, :, :])
            wp = const.tile([64, HPC, C], BF16, tag="wp")
            nc.sync.dma_start(out=wp, in_=wp_d[:, :, :])

            # Prime the natural_log_exp ACT table set (has BOTH Exp and Ln)
            # so no mid-kernel table switches happen: Ln(0*x + 1) = 0.
            LN = mybir.ActivationFunctionType.Ln
            dummy = small.tile([1, 1], F32, tag="dummy")
            nc.scalar.activation(dummy, bqk[0:1, 0:1], LN, scale=0.0, bias=1.0)

            V = []
            for hl in range(HPC):
                vt = vpool.tile([128, 16, 128], BF16, tag=f"v{hl}")
                nc.sync.dma_start(out=vt[:, :, 64:128], in_=ones_d[:, :, :])
                V.append(vt)
            AOUT = [aout.tile([64, N], BF16, tag=f"a{hl}", name=f"aout{hl}")
                    for hl in range(HPC)]
            ANRM = [aout.tile([64, N], BF16, tag=f"an{hl}", name=f"anrm{hl}")
                    for hl in range(HPC)]
            DEN = [aout.tile([64, N], F32, tag=f"dn{hl}", name=f"den{hl}")
                    for hl in range(HPC)]

            with tc.tile_pool(name="psA", bufs=2, space="PSUM") as psA:
                # ---- V projection (all 3 heads at once) ----
                for tt in range(16):
                    psv = psA.tile([128, 256], F32, tag="mm")
                    for kt in range(KT):
                        nc.tensor.matmul(
                            psv,
                            xT[:, kt, tt * 128:(tt + 1) * 128],
                            wv[:, kt, :],
                            start=(kt == 0), stop=(kt == KT - 1),
                        )
                    for hl in range(HPC):
                        nc.vector.tensor_add(
                            V[hl][:, tt, 0:64],
                            psv[:, hl * 64:(hl + 1) * 64],
                            bv[:, hl * 64:(hl + 1) * 64],
                        )

                def qku_steps(hl):
                    """Qa/Ka build for head hl as ~64 atomic steps (1 matmul
                    or 1 DVE/DMA op each), interleaved into the previous
                    head's attention to fill PE exp-shadow slack."""
                    Qa = work.tile([AUG, N], BF16, tag="qaug", name=f"qa{hl}")
                    Ka = work.tile([AUG, N], BF16, tag="kaug", name=f"ka{hl}")
                    st_d = ustage.tile([D, N], BF16, tag="ud", name=f"ud{hl}")
                    st_h = ustage.tile([HH, N], BF16, tag="uh", name=f"uh{hl}")
                    st_w = ustage.tile([WW, N], BF16, tag="uw", name=f"uw{hl}")
                    steps = []
                    steps.append(lambda: nc.sync.dma_start(
                        out=Ka[64:104, :], in_=oneh_d[:, :]))
                    cur = {}

                    def qk_mm(qs, kt):
                        sl = slice(qs * 512, (qs + 1) * 512)
                        if kt == 0:
                            cur['pqk'] = psA.tile([128, 512], F32, tag="mm",
                                                  name=f"pqk{hl}_{qs}")
                        nc.tensor.matmul(
                            cur['pqk'], wqk[:, hl, kt, :], xT[:, kt, sl],
                            start=(kt == 0), stop=(kt == KT - 1))

                    def qk_fin(qs):
                        sl = slice(qs * 512, (qs + 1) * 512)
                        pqk = cur['pqk']
                        nc.vector.tensor_scalar_add(
                            Qa[0:64, sl], pqk[0:64, :], bqk[0:64, hl:hl + 1])
                        nc.vector.tensor_scalar_add(
                            Ka[0:64, sl], pqk[64:128, :], bqk[64:128, hl:hl + 1])
                    for qs in range(4):
                        for kt in range(KT):
                            steps.append(lambda qs=qs, kt=kt: qk_mm(qs, kt))
                        steps.append(lambda qs=qs: qk_fin(qs))

                    def ud_mm(d0):
                        pu = psA.tile([D, 2, 256], F32, tag="mm",
                                      name=f"pud{hl}_{d0}")
                        cur['pud'] = pu
                        for s in range(2):
                            nc.tensor.matmul(
                                pu[:, s, :], rd[:, d0 + s, :],
                                Qa[0:64, (d0 + s) * 256:(d0 + s + 1) * 256],
                                start=True, stop=True)

                    def ud_cp(d0):
                        nc.vector.tensor_copy(
                            st_d[:, d0 * 256:(d0 + 2) * 256],
                            cur['pud'].rearrange("p s n -> p (s n)"))
                    for d0 in range(0, D, 2):
                        steps.append(lambda d0=d0: ud_mm(d0))
                        steps.append(lambda d0=d0: ud_cp(d0))
                    steps.append(lambda: nc.sync.dma_start(
                        out=Qa[64:72, :], in_=st_d))

                    q_h = Qa[0:64, :].rearrange(
                        "p (d h w) -> p d h w", d=D, h=HH, w=WW)
                    u_h = st_h.rearrange(
                        "p (d h w) -> p d h w", d=D, h=HH, w=WW)

                    def uh_mm(h0, s0):
                        if s0 == 0:
                            cur['puh'] = psA.tile([HH, 4, D, WW], F32,
                                                  tag="mm", name=f"puh{hl}_{h0}")
                        for s in (s0, s0 + 1):
                            nc.tensor.matmul(
                                cur['puh'][:, s, :, :], rh[:, h0 + s, :],
                                q_h[:, :, h0 + s, :],
                                start=True, stop=True)

                    def uh_cp(h0):
                        nc.vector.tensor_copy(
                            u_h[:, :, h0:h0 + 4, :],
                            cur['puh'].rearrange("p s d w -> p d s w"))
                    for h0 in range(0, HH, 4):
                        steps.append(lambda h0=h0: uh_mm(h0, 0))
                        steps.append(lambda h0=h0: uh_mm(h0, 2))
                        steps.append(lambda h0=h0: uh_cp(h0))
                    steps.append(lambda: nc.sync.dma_start(
                        out=Qa[72:88, :], in_=st_h))

                    q_w = Qa[0:64, :].rearrange("p (a w) -> p a w", w=WW)
                    u_w = st_w.rearrange("p (a w) -> p a w", w=WW)

                    def uw_mm(w0, s0):
                        if s0 == 0:
                            cur['puw'] = psA.tile([WW, 4, D * HH], F32,
                                                  tag="mm", name=f"puw{hl}_{w0}")
                        for s in (s0, s0 + 1):
                            nc.tensor.matmul(
                                cur['puw'][:, s, :], rw[:, w0 + s, :],
                                q_w[:, :, w0 + s],
                                start=True, stop=True)

                    def uw_cp(w0):
                        nc.vector.tensor_copy(
                            u_w[:, :, w0:w0 + 4],
                            cur['puw'].rearrange("p s a -> p a s"))
                    for w0 in range(0, WW, 4):
                        steps.append(lambda w0=w0: uw_mm(w0, 0))
                        steps.append(lambda w0=w0: uw_mm(w0, 2))
                        steps.append(lambda w0=w0: uw_cp(w0))
                    steps.append(lambda: nc.sync.dma_start(
                        out=Qa[88:104, :], in_=st_w))
                    return (Qa, Ka), steps

                with (
                    tc.tile_pool(name="pss", bufs=2, space="PSUM") as pss,
                    tc.tile_pool(name="pso", bufs=1, space="PSUM") as pso,
                ):
                    # prologue: head 0's Qa/Ka built up front
                    (Qa0, Ka0), steps0 = qku_steps(0)
                    for s in steps0:
                        s()
                    cur = (Qa0, Ka0)
                    pending = []
                    for hl in range(HPC):
                        Qa, Ka = cur
                        if hl + 1 < HPC:
                            nxt, pending = qku_steps(hl + 1)
                        else:
                            nxt, pending = None, []
                        step_i = 0
                        for half in range(2):
                            qo = half * 1024
                            ot = pso.tile([128, 1024], F32, tag="o",
                                          name=f"ot{hl}_{half}")
                            prev = None
                            for kt in range(16):
                                st = pss.tile([128, 1024], F32, tag="s",
                                              name=f"st{hl}_{half}_{kt}")
                                for j in range(2):
                                    nc.tensor.matmul(
                                        st[:, j * 512:(j + 1) * 512],
                                        Ka[:, kt * 128:(kt + 1) * 128],
                                        Qa[:, qo + j * 512: qo + (j + 1) * 512],
                                        start=True, stop=True)
                                    if step_i < len(pending):
                                        pending[step_i]()
                                        step_i += 1
                                if prev is not None:
                                    pkt, ppt = prev
                                    for j in range(2):
                                        nc.tensor.matmul(
                                            ot[:, j * 512:(j + 1) * 512],
                                            V[hl][:, pkt, :],
                                            ppt[:, j * 512:(j + 1) * 512],
                                            start=(pkt == 0), stop=False)
                                pt = ptp.tile([128, 1024], BF16, tag="pt",
                                              name=f"pt{hl}_{half}_{kt}")
                                nc.scalar.activation(pt, st, EXP)
                                prev = (kt, pt)
                            pkt, ppt = prev
                            for j in range(2):
                                nc.tensor.matmul(
                                    ot[:, j * 512:(j + 1) * 512],
                                    V[hl][:, pkt, :],
                                    ppt[:, j * 512:(j + 1) * 512],
                                    start=False, stop=True)
                            nc.vector.tensor_copy(
                                AOUT[hl][:, qo:qo + 1024], ot[0:64, :])
                            nc.vector.tensor_copy(
                                DEN[hl][:, qo:qo + 1024], ot[64:128, :])
                        # drain any leftover steps for the next head
                        while step_i < len(pending):
                            pending[step_i]()
                            step_i += 1
                        cur = nxt

            # ---- tail normalize (off critical path) + projection ----
            for hl in range(HPC):
                lnd = small.tile([64, N], F32, tag="lnd")
                nc.scalar.activation(lnd, DEN[hl], LN)
                rcp = small.tile([64, N], F32, tag="rcp")
                nc.scalar.activation(rcp, lnd, EXP, scale=-1.0)
                nc.vector.tensor_mul(ANRM[hl], AOUT[hl][0:64, :], rcp)
            with tc.tile_pool(name="psp", bufs=2, space="PSUM") as psp:
                for tt in range(16):
                    pp = psp.tile([128, C], F32, tag="pp")
                    for hl in range(HPC):
                        nc.tensor.matmul(
                            pp[:, 0:512],
                            ANRM[hl][:, tt * 128:(tt + 1) * 128],
                            wp[:, hl, 0:512],
                            start=(hl == 0), stop=(hl == HPC - 1))
                        nc.tensor.matmul(
                            pp[:, 512:768],
                            ANRM[hl][:, tt * 128:(tt + 1) * 128],
                            wp[:, hl, 512:768],
                            start=(hl == 0), stop=(hl == HPC - 1))
                    osb = outp.tile([128, C], F32, tag="osb")
                    nc.vector.tensor_copy(osb, pp)
                    nc.sync.dma_start(
                        out=out_d[tt * 128:(tt + 1) * 128, :], in_=osb)
    nc.compile()
    return nc


def _get_rel(sz, t):
    coords = np.arange(sz)[:, None] - np.arange(sz)[None, :] + sz - 1
    return t[coords]


def kernel(x, qkv_w, qkv_b, proj_w, proj_b,
           rel_pos_d, rel_pos_h, rel_pos_w):
    global LAST_RESULTS
    x = np.ascontiguousarray(np.asarray(x, dtype=np.float32))
    qkv_w = np.asarray(qkv_w, dtype=np.float32)
    qkv_b = np.asarray(qkv_b, dtype=np.float32)
    proj_w = np.asarray(proj_w, dtype=np.float32)
    proj_b = np.asarray(proj_b, dtype=np.float32)
    rel_pos_d = np.asarray(rel_pos_d, dtype=np.float32)
    rel_pos_h = np.asarray(rel_pos_h, dtype=np.float32)
    rel_pos_w = np.asarray(rel_pos_w, dtype=np.float32)

    xf = x.reshape(B, N, C)

    # shared host-prepped tables
    rd_t = np.ascontiguousarray(
        (_get_rel(D, rel_pos_d) / SCALE).transpose(2, 0, 1), np.float32)
    rh_t = np.ascontiguousarray(
        (_get_rel(HH, rel_pos_h) / SCALE).transpose(2, 0, 1), np.float32)
    rw_t = np.ascontiguousarray(
        (_get_rel(WW, rel_pos_w) / SCALE).transpose(2, 0, 1), np.float32)
    dd = np.arange(N) // (HH * WW)
    hh = (np.arange(N) // WW) % HH
    ww = np.arange(N) % WW
    onehot = np.zeros((40, N), np.float32)
    onehot[dd, np.arange(N)] = 1.0
    onehot[D + hh, np.arange(N)] = 1.0
    onehot[D + HH + ww, np.arange(N)] = 1.0

    xT_b = []
    for b in range(B):
        xT = xf[b].T.reshape(KT, 128, N).transpose(1, 0, 2)
        xT_b.append(np.ascontiguousarray(xT))

    in_maps = []
    for core in range(NCORES):
        b = core // 4
        heads = [HPC * (core % 4) + i for i in range(HPC)]
        wqk_l, bqk_l, wv_l, bv_l, wp_l = [], [], [], [], []
        for h in heads:
            wqk_l.append(np.concatenate([
                qkv_w[:, h * HD:(h + 1) * HD] * SCALE,
                qkv_w[:, C + h * HD: C + (h + 1) * HD]], axis=1))
            bqk_l.append(np.concatenate([
                qkv_b[h * HD:(h + 1) * HD] * SCALE,
                qkv_b[C + h * HD: C + (h + 1) * HD]]))
            wv_l.append(qkv_w[:, 2 * C + h * HD: 2 * C + (h + 1) * HD])
            bv_l.append(qkv_b[2 * C + h * HD: 2 * C + (h + 1) * HD])
            wp_l.append(proj_w[h * HD:(h + 1) * HD, :])
        w_qk = np.stack(wqk_l)                       # [3, 768, 128]
        w_qk = np.ascontiguousarray(
            w_qk.reshape(HPC, KT, 128, 128).transpose(2, 0, 1, 3))
        b_qk = np.ascontiguousarray(np.stack(bqk_l).T)   # [128, 3]
        w_v = np.concatenate(wv_l + [np.zeros((C, 64), np.float32)], axis=1)
        w_v = np.ascontiguousarray(
            w_v.reshape(KT, 128, 256).transpose(1, 0, 2))
        b_v = np.broadcast_to(
            np.concatenate(bv_l + [np.zeros(64, np.float32)])[None, :],
            (128, 256))
        w_p = np.ascontiguousarray(np.stack(wp_l).transpose(1, 0, 2))

        bf = ml_dtypes.bfloat16
        in_maps.append({
            "xT": xT_b[b].astype(bf),
            "w_qk": w_qk.astype(bf),
            "b_qk": b_qk,
            "w_v": w_v.astype(bf),
            "b_v": np.ascontiguousarray(b_v),
            "rd_t": rd_t.astype(bf),
            "rh_t": rh_t.astype(bf),
            "rw_t": rw_t.astype(bf),
            "onehot": onehot.astype(bf),
            "w_p": w_p.astype(bf),
            "ones_col": np.ones((128, 16, 64), bf),
        })

    nc = _build_bass()
    res = run_bass_kernel_spmd(nc, in_maps, core_ids=list(range(NCORES)))
    LAST_RESULTS = res

    out = np.zeros((B, N, C), np.float32)
    for core in range(NCORES):
        out[core // 4] += res.results[core]["out_part"]
    out += proj_b
    return out.reshape(B, D, HH, WW, C)


# revision 22
# speedup vs baseline: 1.1967x; 1.0610x over previous
"""Trainium2 Bass kernel for 3D attention with decomposed rel-pos bias.

Problem: nn_Attention3D (B=2, D=8, H=16, W=16, C=768, nh=12, hd=64).

Strategy
--------
- Shard the 24 (batch, head) pairs as 3 per core x 8 cores; each core's 3
  heads share one batch, so the core can emit a single per-batch partial of
  the output projection (summed on host across the 4 cores per batch).
- Rel-pos bias is folded into the QK^T matmul by augmenting the contraction
  dim: score[q,k] = [scale*q, U_d(q), U_h(q), U_w(q)] . [k, 1hot_d(k),
  1hot_h(k), 1hot_w(k)], with U_d = q @ Rd[d(q)]^T etc.  64+8+16+16 = 104
  <= 128 partitions, so the whole biased logit matrix costs one matmul pass
  and zero elementwise broadcast adds.
- Softmax without max-subtraction (logits are ~N(0, 0.3), |logit| < 3).
- Softmax denominator folded into the PV matmul as a ones-column of V.
- All matmuls in float32r (full PE speed at N>=256), fp32 data everywhere.
"""

import os
import sys

import numpy as np
import ml_dtypes

for _p in ("/opt/trn_rl_repo", "/root/.axon_site/_ro/trn_rl_repo"):
    if os.path.isdir(_p) and _p not in sys.path:
        sys.path.append(_p)

import concourse.bass as bass
import concourse.mybir as mybir
from concourse import bacc
from concourse.bass_utils import run_bass_kernel_spmd
from concourse.tile import TileContext

F32 = mybir.dt.float32
F32R = mybir.dt.float32r
BF16 = mybir.dt.bfloat16
EXP = mybir.ActivationFunctionType.Exp

B, D, HH, WW, C = 2, 8, 16, 16, 768
NH, HD = 12, 64
N = D * HH * WW            # 2048
SCALE = HD ** -0.5
NCORES = 8
HPC = 3                    # heads per core
KT = C // 128              # 6 contraction tiles for the input projections
AUG = HD + D + HH + WW     # 104 augmented contraction

LAST_RESULTS = None        # stashed BassKernelResults for test.py


def _build_bass():
    nc = bacc.Bacc()
    xT_d = nc.dram_tensor("xT", [128, KT, N], BF16, kind="ExternalInput")
    wqk_d = nc.dram_tensor("w_qk", [128, HPC, KT, 128], BF16, kind="ExternalInput")
    bqk_d = nc.dram_tensor("b_qk", [128, HPC], F32, kind="ExternalInput")
    wv_d = nc.dram_tensor("w_v", [128, KT, 256], BF16, kind="ExternalInput")
    bv_d = nc.dram_tensor("b_v", [128, 256], F32, kind="ExternalInput")
    rd_d = nc.dram_tensor("rd_t", [64, D, D], BF16, kind="ExternalInput")
    rh_d = nc.dram_tensor("rh_t", [64, HH, HH], BF16, kind="ExternalInput")
    rw_d = nc.dram_tensor("rw_t", [64, WW, WW], BF16, kind="ExternalInput")
    oneh_d = nc.dram_tensor("onehot", [40, N], BF16, kind="ExternalInput")
    wp_d = nc.dram_tensor("w_p", [64, HPC, C], BF16, kind="ExternalInput")
    ones_d = nc.dram_tensor("ones_col", [128, 16, 64], BF16, kind="ExternalInput")
    out_d = nc.dram_tensor("out_part", [N, C], F32, kind="ExternalOutput")

    with TileContext(nc) as tc:
        with (
            tc.tile_pool(name="const", bufs=1) as const,
            tc.tile_pool(name="work", bufs=2) as work,
            tc.tile_pool(name="vpool", bufs=1) as vpool,
            tc.tile_pool(name="ptp", bufs=2) as ptp,
            tc.tile_pool(name="aout", bufs=1) as aout,
            tc.tile_pool(name="ustage", bufs=2) as ustage,
            tc.tile_pool(name="small", bufs=1) as small,
            tc.tile_pool(name="outp", bufs=2) as outp,
        ):
            # ---- constants: small attention-critical DMAs first ----
            wv = const.tile([128, KT, 256], BF16, tag="wv")
            nc.sync.dma_start(out=wv, in_=wv_d[:, :, :])
            bv = const.tile([128, 256], F32, tag="bv")
            nc.sync.dma_start(out=bv, in_=bv_d[:, :])
            bqk = const.tile([128, HPC], F32, tag="bqk")
            nc.sync.dma_start(out=bqk, in_=bqk_d[:, :])
            rd = const.tile([64, D, D], BF16, tag="rd")
            nc.sync.dma_start(out=rd, in_=rd_d[:, :, :])
            rh = const.tile([64, HH, HH], BF16, tag="rh")
            nc.sync.dma_start(out=rh, in_=rh_d[:, :, :])
            rw = const.tile([64, WW, WW], BF16, tag="rw")
            nc.sync.dma_start(out=rw, in_=rw_d[:, :, :])
            xT = const.tile([128, KT, N], BF16, tag="xT")
            for tt in range(16):
                sl = slice(tt * 128, (tt + 1) * 128)
                nc.sync.dma_start(out=xT[:, :, sl], in_=xT_d[:, :, sl])
            wqk = const.tile([128, HPC, KT, 128], BF16, tag="wqk")
            nc.sync.dma_start(out=wqk, in_=wqk_d[:, :, :, :])
            wp = const.tile([64, HPC, C], BF16, tag="wp")
            nc.sync.dma_start(out=wp, in_=wp_d[:, :, :])

            # Prime the natural_log_exp ACT table set (has BOTH Exp and Ln)
            # so no mid-kernel table switches happen: Ln(0*x + 1) = 0.
            LN = mybir.ActivationFunctionType.Ln
            dummy = small.tile([1, 1], F32, tag="dummy")
            nc.scalar.activation(dummy, bqk[0:1, 0:1], LN, scale=0.0, bias=1.0)

            V = []
            for hl in range(HPC):
                vt = vpool.tile([128, 16, 128], BF16, tag=f"v{hl}")
                nc.sync.dma_start(out=vt[:, :, 64:128], in_=ones_d[:, :, :])
                V.append(vt)
            AOUT = [aout.tile([64, N], BF16, tag=f"a{hl}", name=f"aout{hl}")
                    for hl in range(HPC)]
            ANRM = [aout.tile([64, N], BF16, tag=f"an{hl}", name=f"anrm{hl}")
                    for hl in range(HPC)]
            DEN = [aout.tile([64, N], F32, tag=f"dn{hl}", name=f"den{hl}")
                    for hl in range(HPC)]

            with tc.tile_pool(name="psA", bufs=2, space="PSUM") as psA:
                # ---- V projection (all 3 heads at once) ----
                for tt in range(16):
                    psv = psA.tile([128, 256], F32, tag="mm")
                    for kt in range(KT):
                        nc.tensor.matmul(
                            psv,
                            xT[:, kt, tt * 128:(tt + 1) * 128],
                            wv[:, kt, :],
                            start=(kt == 0), stop=(kt == KT - 1),
                        )
                    for hl in range(HPC):
                        nc.vector.tensor_add(
                            V[hl][:, tt, 0:64],
                            psv[:, hl * 64:(hl + 1) * 64],
                            bv[:, hl * 64:(hl + 1) * 64],
                        )

                def qku_steps(hl):
                    """Qa/Ka build for head hl as ~64 atomic steps (1 matmul
                    or 1 DVE/DMA op each), interleaved into the previous
                    head's attention to fill PE exp-shadow slack."""
                    Qa = work.tile([AUG, N], BF16, tag="qaug", name=f"qa{hl}")
                    Ka = work.tile([AUG, N], BF16, tag="kaug", name=f"ka{hl}")
                    st_d = ustage.tile([D, N], BF16, tag="ud", name=f"ud{hl}")
                    st_h = ustage.tile([HH, N], BF16, tag="uh", name=f"uh{hl}")
                    st_w = ustage.tile([WW, N], BF16, tag="uw", name=f"uw{hl}")
                    steps = []
                    steps.append(lambda: nc.sync.dma_start(
                        out=Ka[64:104, :], in_=oneh_d[:, :]))
                    cur = {}

                    def qk_mm(qs, kt):
                        sl = slice(qs * 512, (qs + 1) * 512)
                        if kt == 0:
                            cur['pqk'] = psA.tile([128, 512], F32, tag="mm",
                                                  name=f"pqk{hl}_{qs}")
                        nc.tensor.matmul(
                            cur['pqk'], wqk[:, hl, kt, :], xT[:, kt, sl],
                            start=(kt == 0), stop=(kt == KT - 1))

                    def qk_fin(qs):
                        sl = slice(qs * 512, (qs + 1) * 512)
                        pqk = cur['pqk']
                        nc.vector.tensor_scalar_add(
                            Qa[0:64, sl], pqk[0:64, :], bqk[0:64, hl:hl + 1])
                        nc.vector.tensor_scalar_add(
                            Ka[0:64, sl], pqk[64:128, :], bqk[64:128, hl:hl + 1])
                    for qs in range(4):
                        for kt in range(KT):
                            steps.append(lambda qs=qs, kt=kt: qk_mm(qs, kt))
                        steps.append(lambda qs=qs: qk_fin(qs))

                    def ud_mm(d0):
                        pu = psA.tile([D, 2, 256], F32, tag="mm",
                                      name=f"pud{hl}_{d0}")
                        cur['pud'] = pu
                        for s in range(2):
                            nc.tensor.matmul(
                                pu[:, s, :], rd[:, d0 + s, :],
                                Qa[0:64, (d0 + s) * 256:(d0 + s + 1) * 256],
                                start=True, stop=True)

                    def ud_cp(d0):
                        nc.vector.tensor_copy(
                            st_d[:, d0 * 256:(d0 + 2) * 256],
                            cur['pud'].rearrange("p s n -> p (s n)"))
                    for d0 in range(0, D, 2):
                        steps.append(lambda d0=d0: ud_mm(d0))
                        steps.append(lambda d0=d0: ud_cp(d0))
                    steps.append(lambda: nc.sync.dma_start(
                        out=Qa[64:72, :], in_=st_d))

                    q_h = Qa[0:64, :].rearrange(
                        "p (d h w) -> p d h w", d=D, h=HH, w=WW)
                    u_h = st_h.rearrange(
                        "p (d h w) -> p d h w", d=D, h=HH, w=WW)

                    def uh_mm(h0, s0):
                        if s0 == 0:
                            cur['puh'] = psA.tile([HH, 4, D, WW], F32,
                                                  tag="mm", name=f"puh{hl}_{h0}")
                        for s in (s0, s0 + 1):
                            nc.tensor.matmul(
                                cur['puh'][:, s, :, :], rh[:, h0 + s, :],
                                q_h[:, :, h0 + s, :],
                                start=True, stop=True)

                    def uh_cp(h0):
                        nc.vector.tensor_copy(
                            u_h[:, :, h0:h0 + 4, :],
                            cur['puh'].rearrange("p s d w -> p d s w"))
                    for h0 in range(0, HH, 4):
                        steps.append(lambda h0=h0: uh_mm(h0, 0))
                        steps.append(lambda h0=h0: uh_mm(h0, 2))
                        steps.append(lambda h0=h0: uh_cp(h0))
                    steps.append(lambda: nc.sync.dma_start(
                        out=Qa[72:88, :], in_=st_h))

                    q_w = Qa[0:64, :].rearrange("p (a w) -> p a w", w=WW)
                    u_w = st_w.rearrange("p (a w) -> p a w", w=WW)

                    def uw_mm(w0, s0):
                        if s0 == 0:
                            cur['puw'] = psA.tile([WW, 4, D * HH], F32,
                                                  tag="mm", name=f"puw{hl}_{w0}")
                        for s in (s0, s0 + 1):
                            nc.tensor.matmul(
                                cur['puw'][:, s, :], rw[:, w0 + s, :],
                                q_w[:, :, w0 + s],
                                start=True, stop=True)

                    def uw_cp(w0):
                        nc.vector.tensor_copy(
                            u_w[:, :, w0:w0 + 4],
                            cur['puw'].rearrange("p s a -> p a s"))
                    for w0 in range(0, WW, 4):
                        steps.append(lambda w0=w0: uw_mm(w0, 0))
                        steps.append(lambda w0=w0: uw_mm(w0, 2))
                        steps.append(lambda w0=w0: uw_cp(w0))
                    steps.append(lambda: nc.sync.dma_start(
                        out=Qa[88:104, :], in_=st_w))
                    return (Qa, Ka), steps

                with (
                    tc.tile_pool(name="pss", bufs=2, space="PSUM") as pss,
                    tc.tile_pool(name="pso", bufs=1, space="PSUM") as pso,
                ):
                    # prologue: head 0's Qa/Ka built up front
                    (Qa0, Ka0), steps0 = qku_steps(0)
                    for s in steps0:
                        s()
                    cur = (Qa0, Ka0)
                    pending = []
                    for hl in range(HPC):
                        Qa, Ka = cur
                        if hl + 1 < HPC:
                            nxt, pending = qku_steps(hl + 1)
                        else:
                            nxt, pending = None, []
                        step_i = 0
                        for half in range(2):
                            qo = half * 1024
                            ot = pso.tile([128, 1024], F32, tag="o",
                                          name=f"ot{hl}_{half}")
                            prev = None
                            for kt in range(16):
                                st = pss.tile([128, 1024], F32, tag="s",
                                              name=f"st{hl}_{half}_{kt}")
                                for j in range(2):
                                    nc.tensor.matmul(
                                        st[:, j * 512:(j + 1) * 512],
                                        Ka[:, kt * 128:(kt + 1) * 128],
                                        Qa[:, qo + j * 512: qo + (j + 1) * 512],
                                        start=True, stop=True)
                                    if step_i < len(pending):
                                        pending[step_i]()
                                        step_i += 1
                                if prev is not None:
                                    pkt, ppt = prev
                                    for j in range(2):
                                        nc.tensor.matmul(
                                            ot[:, j * 512:(j + 1) * 512],
                                            V[hl][:, pkt, :],
                                            ppt[:, j * 512:(j + 1) * 512],
                                            start=(pkt == 0), stop=False)
                                pt = ptp.tile([128, 1024], BF16, tag="pt",
                                              name=f"pt{hl}_{half}_{kt}")
                                nc.scalar.activation(pt, st, EXP)
                                prev = (kt, pt)
                            pkt, ppt = prev
                            for j in range(2):
                                nc.tensor.matmul(
                                    ot[:, j * 512:(j + 1) * 512],
                                    V[hl][:, pkt, :],
                                    ppt[:, j * 512:(j + 1) * 512],
                                    start=False, stop=True)
                            nc.vector.tensor_copy(
                                AOUT[hl][:, qo:qo + 1024], ot[0:64, :])
                            nc.vector.tensor_copy(
                                DEN[hl][:, qo:qo + 1024], ot[64:128, :])
                        # normalize this head now: Ln/Exp tables are
                        # resident, ACT has a bubble at the head boundary
                        lnd = small.tile([64, N], F32, tag="lnd",
                                         name=f"lnd{hl}")
                        nc.scalar.activation(lnd, DEN[hl], LN)
                        rcp = small.tile([64, N], F32, tag="rcp",
                                         name=f"rcp{hl}")
                        nc.scalar.activation(rcp, lnd, EXP, scale=-1.0)
                        nc.vector.tensor_mul(
                            ANRM[hl], AOUT[hl][0:64, :], rcp)
                        # drain any leftover steps for the next head
                        while step_i < len(pending):
                            pending[step_i]()
                            step_i += 1
                        cur = nxt

            # ---- projection ----
            with tc.tile_pool(name="psp", bufs=2, space="PSUM") as psp:
                for tt in range(16):
                    pp = psp.tile([128, C], F32, tag="pp")
                    for hl in range(HPC):
                        nc.tensor.matmul(
                            pp[:, 0:512],
                            ANRM[hl][:, tt * 128:(tt + 1) * 128],
                            wp[:, hl, 0:512],
                            start=(hl == 0), stop=(hl == HPC - 1))
                        nc.tensor.matmul(
                            pp[:, 512:768],
                            ANRM[hl][:, tt * 128:(tt + 1) * 128],
                            wp[:, hl, 512:768],
                            start=(hl == 0), stop=(hl == HPC - 1))
                    osb = outp.tile([128, C], F32, tag="osb")
                    nc.vector.tensor_copy(osb, pp)
                    nc.sync.dma_start(
                        out=out_d[tt * 128:(tt + 1) * 128, :], in_=osb)
    nc.compile()
    return nc


def _get_rel(sz, t):
    coords = np.arange(sz)[:, None] - np.arange(sz)[None, :] + sz - 1
    return t[coords]


def kernel(x, qkv_w, qkv_b, proj_w, proj_b,
           rel_pos_d, rel_pos_h, rel_pos_w):
    global LAST_RESULTS
    x = np.ascontiguousarray(np.asarray(x, dtype=np.float32))
    qkv_w = np.asarray(qkv_w, dtype=np.float32)
    qkv_b = np.asarray(qkv_b, dtype=np.float32)
    proj_w = np.asarray(proj_w, dtype=np.float32)
    proj_b = np.asarray(proj_b, dtype=np.float32)
    rel_pos_d = np.asarray(rel_pos_d, dtype=np.float32)
    rel_pos_h = np.asarray(rel_pos_h, dtype=np.float32)
    rel_pos_w = np.asarray(rel_pos_w, dtype=np.float32)

    xf = x.reshape(B, N, C)

    # shared host-prepped tables
    rd_t = np.ascontiguousarray(
        (_get_rel(D, rel_pos_d) / SCALE).transpose(2, 0, 1), np.float32)
    rh_t = np.ascontiguousarray(
        (_get_rel(HH, rel_pos_h) / SCALE).transpose(2, 0, 1), np.float32)
    rw_t = np.ascontiguousarray(
        (_get_rel(WW, rel_pos_w) / SCALE).transpose(2, 0, 1), np.float32)
    dd = np.arange(N) // (HH * WW)
    hh = (np.arange(N) // WW) % HH
    ww = np.arange(N) % WW
    onehot = np.zeros((40, N), np.float32)
    onehot[dd, np.arange(N)] = 1.0
    onehot[D + hh, np.arange(N)] = 1.0
    onehot[D + HH + ww, np.arange(N)] = 1.0

    xT_b = []
    for b in range(B):
        xT = xf[b].T.reshape(KT, 128, N).transpose(1, 0, 2)
        xT_b.append(np.ascontiguousarray(xT))

    in_maps = []
    for core in range(NCORES):
        b = core // 4
        heads = [HPC * (core % 4) + i for i in range(HPC)]
        wqk_l, bqk_l, wv_l, bv_l, wp_l = [], [], [], [], []
        for h in heads:
            wqk_l.append(np.concatenate([
                qkv_w[:, h * HD:(h + 1) * HD] * SCALE,
                qkv_w[:, C + h * HD: C + (h + 1) * HD]], axis=1))
            bqk_l.append(np.concatenate([
                qkv_b[h * HD:(h + 1) * HD] * SCALE,
                qkv_b[C + h * HD: C + (h + 1) * HD]]))
            wv_l.append(qkv_w[:, 2 * C + h * HD: 2 * C + (h + 1) * HD])
            bv_l.append(qkv_b[2 * C + h * HD: 2 * C + (h + 1) * HD])
            wp_l.append(proj_w[h * HD:(h + 1) * HD, :])
        w_qk = np.stack(wqk_l)                       # [3, 768, 128]
        w_qk = np.ascontiguousarray(
            w_qk.reshape(HPC, KT, 128, 128).transpose(2, 0, 1, 3))
        b_qk = np.ascontiguousarray(np.stack(bqk_l).T)   # [128, 3]
        w_v = np.concatenate(wv_l + [np.zeros((C, 64), np.float32)], axis=1)
        w_v = np.ascontiguousarray(
            w_v.reshape(KT, 128, 256).transpose(1, 0, 2))
        b_v = np.broadcast_to(
            np.concatenate(bv_l + [np.zeros(64, np.float32)])[None, :],
            (128, 256))
        w_p = np.ascontiguousarray(np.stack(wp_l).transpose(1, 0, 2))

        bf = ml_dtypes.bfloat16
        in_maps.append({
            "xT": xT_b[b].astype(bf),
            "w_qk": w_qk.astype(bf),
            "b_qk": b_qk,
            "w_v": w_v.astype(bf),
            "b_v": np.ascontiguousarray(b_v),
            "rd_t": rd_t.astype(bf),
            "rh_t": rh_t.astype(bf),
            "rw_t": rw_t.astype(bf),
            "onehot": onehot.astype(bf),
            "w_p": w_p.astype(bf),
            "ones_col": np.ones((128, 16, 64), bf),
        })

    nc = _build_bass()
    res = run_bass_kernel_spmd(nc, in_maps, core_ids=list(range(NCORES)))
    LAST_RESULTS = res

    out = np.zeros((B, N, C), np.float32)
    for core in range(NCORES):
        out[core // 4] += res.results[core]["out_part"]
    out += proj_b
    return out.reshape(B, D, HH, WW, C)
